# revision 57
# baseline (speedup 1.0000x reference)
"""ALiBi positional-bias kernel for 8 TRN2 NeuronCores.

out[b,h,i,j] = scores[b,h,i,j] + bias[h,i,j]
bias[h,i,j] = -inf                                   for j > i
            = d*A_h + B_h + P(d)*C_h  (d = i-j)      for j <= i
  A_h = (base_h*slope_scaling_h + slope_adjustment_h) * math_bias_scale_h
  B_h = sequential_bias_h
  C_h = hierarchical_bias_h
  P(d) = 0.1 (d in fib), 0.05 (d in pows), 0.02 (d%10==0), else 0

The bias is Toeplitz, so its 128x128 tiles depend only on the tile-diagonal
offset c = ti - tj: 16 distinct tiles per head.  Each core owns 2 heads x
2 batches; it builds the 16 bias tiles per head once in SBUF (as one
[128, 2048] strip, tile-diag c stored at column-tile 15-c so any causal row
strip is one contiguous slice), then streams score row-strips through a
single fused tensor_tensor add.  The strictly-upper-triangular region is
constant -inf and is written straight from a preset SBUF tile without ever
reading the scores there.
"""

import math

import numpy as np

B, H, S = 2, 16, 2048
P = 128
NT = S // P
NCORES = 8
HC = H // NCORES  # heads per core

VEC_NAMES = [
    "slope_adjustment",
    "slope_scaling",
    "math_bias_scale",
    "sequential_bias",
    "hierarchical_bias",
]


def _base_slopes(n):
    def pow2_slopes(m):
        start = 2 ** (-(2 ** (-(math.log2(m) - 3))))
        return [start * start**i for i in range(m)]

    if math.log2(n).is_integer():
        return pow2_slopes(n)
    c = 2 ** math.floor(math.log2(n))
    return pow2_slopes(c) + _base_slopes(2 * c)[0::2][: n - c]


def _hier_1d(n):
    d = np.arange(n)
    fib = np.array([1, 2, 3, 5, 8, 13, 21])
    pows = np.array([4, 16, 64, 256])
    return np.where(
        np.isin(d, fib), 0.1, np.where(np.isin(d, pows), 0.05, np.where(d % 10 == 0, 0.02, 0.0))
    ).astype(np.float32)


def _const_tiles(s):
    """pfull: [128, s] strip; column-tile m holds tile-diag c = nt-1-m,
    pfull[p, m*128+jj] = P(128*c + p - jj) (0 where d<0; -inf-masked later)."""
    nt = s // P
    hier = _hier_1d(s)
    p_idx = np.arange(P)[:, None]
    jj = np.arange(P)[None, :]
    pfull = np.zeros((P, s), np.float32)
    for m in range(nt):
        c = nt - 1 - m
        d = c * P + p_idx - jj
        pv = hier[np.clip(d, 0, s - 1)]
        pfull[:, m * P : (m + 1) * P] = np.where(d < 0, 0.0, pv)
    return pfull


def build_kernel_body(tc, io, bc, hc, s, bufs=8):
    """Emit the per-core program.  io maps tensor name -> bass.AP."""
    import concourse.mybir as mybir

    nc = tc.nc
    f32 = mybir.dt.float32
    add = mybir.AluOpType.add
    mult = mybir.AluOpType.mult
    nt = s // P

    with (
        tc.tile_pool(name="consts", bufs=1) as cpool,
        tc.tile_pool(name="work", bufs=2) as wpool,
    ):
        # -inf source for the slot-tail fills, first thing on the idle DVE
        inf_src = cpool.tile([P, s - P], f32, tag="inf_src")
        nc.vector.memset(inf_src[:], float("-inf"))

        # dfull[p, m*128+jj] = 128*(nt-1-m) + p - jj, generated on-device
        dfull = cpool.tile([P, s], f32, tag="dfull")
        nc.gpsimd.iota(
            dfull[:],
            pattern=[[-P, nt], [-1, P]],
            base=(nt - 1) * P,
            channel_multiplier=1,
            allow_small_or_imprecise_dtypes=True,
        )
        # warmup touch: absorbs the cold-start HBM receipt latency so the
        # pvec semaphore (which heads the bias-strip chain) fires sooner
        warm = cpool.tile([1, 4], f32, tag="warm")
        nc.sync.dma_start(out=warm[0:1, 0:4], in_=io["pfull"][0:1, 0:4])

        pfull = cpool.tile([P, s], f32, tag="pfull")
        nc.sync.dma_start(out=pfull[:], in_=io["pfull"])

        # packed params pvec = [scaling | seq | hier | base | adj | mscale], one
        # DMA; single_packet so the 48B transfer completes on one engine
        # instead of waiting for all 16 to cycle through
        ta = cpool.tile([1, 6 * hc], f32, tag="ta")
        nc.sync.dma_start(out=ta[0:1, :], in_=io["pvec"][None, :], single_packet=True)

        # abc0 = [A | B | C | ones(128)]; A = (scaling*base + adj) * mscale
        abc0 = cpool.tile([1, 3 * hc + P], f32, tag="abc0")
        nc.vector.tensor_tensor(
            abc0[0:1, 0:hc], ta[0:1, 0:hc], ta[0:1, 3 * hc : 4 * hc], mult
        )
        nc.vector.tensor_tensor(
            abc0[0:1, 0:hc], abc0[0:1, 0:hc], ta[0:1, 4 * hc : 5 * hc], add
        )
        nc.vector.tensor_tensor(
            abc0[0:1, 0:hc], abc0[0:1, 0:hc], ta[0:1, 5 * hc : 6 * hc], mult
        )
        nc.vector.tensor_copy(abc0[0:1, hc : 3 * hc], ta[0:1, hc : 3 * hc])
        nc.vector.memset(abc0[0:1, 3 * hc : 3 * hc + P], 1.0)

        # broadcast [1, 3*hc] -> [128, 3*hc] via ones outer-product on PE
        with tc.tile_pool(name="psum", bufs=1, space="PSUM") as ppool:
            pt = ppool.tile([P, 3 * hc], f32, tag="pt")
            nc.tensor.matmul(
                pt[:],
                abc0[0:1, 3 * hc : 3 * hc + P],
                abc0[0:1, 0 : 3 * hc],
                start=True,
                stop=True,
            )
            abcb = cpool.tile([P, 3 * hc], f32, tag="abcb")
            nc.vector.tensor_copy(abcb[:], pt[:])

        # bias strips per head: kt = D*A + (P*C + B), tile-diag c = nt-1-m
        # stored at column-tile m; row strip for row-tile ti = kt[:, s-w:s].
        # Built back-to-front in two chunks: the diagonal-end columns (the
        # only part early row-tiles read) are ready before the full strip,
        # so the write stream starts several us earlier.
        ch = min(4 * P, s)
        bounds = [(s - ch, s)] + ([(0, s - ch)] if ch < s else [])
        kts = [cpool.tile([P, s], f32, name=f"k{h}", tag=f"k{h}") for h in range(hc)]
        pcs = [wpool.tile([P, s], f32, name=f"pc{h}", tag="pc") for h in range(hc)]
        for lo, hi in bounds:
            for h in range(hc):
                kt, pc = kts[h], pcs[h]
                nc.vector.tensor_scalar(
                    pc[:, lo:hi],
                    pfull[:, lo:hi],
                    abcb[:, 2 * hc + h : 2 * hc + h + 1],
                    abcb[:, hc + h : hc + h + 1],
                    mult,
                    add,
                )
                nc.vector.scalar_tensor_tensor(
                    kt[:, lo:hi],
                    dfull[:, lo:hi],
                    abcb[:, h : h + 1],
                    pc[:, lo:hi],
                    mult,
                    add,
                )
                if hi == s:
                    # diag tile: -inf where jj > p (strictly upper triangle)
                    nc.gpsimd.affine_select(
                        kt[:, s - P : s],
                        kt[:, s - P : s],
                        pattern=[[-1, P]],
                        compare_op=mybir.AluOpType.is_ge,
                        fill=float("-inf"),
                        base=0,
                        channel_multiplier=1,
                    )

        sc = io["scores"]
        out = io["out"]
        # ti-major ascending: causal width w is globally non-decreasing, so
        # each slot's [w:s] tail keeps the -inf laid down at its first use —
        # every output write is one full-width DMA (uniform 8KB descriptors).
        # The tail's full-width reads are prefetched up front so the DMA
        # engines have work during the ramp while the bias strips are built.
        pairs = [(b, h) for h in range(hc) for b in range(bc)]  # h0 first
        sched = [(b, h, ti) for ti in range(nt) for (b, h) in pairs]

        n_pf = min(len(pairs), len(sched))  # prefetch the widest strips
        pf_start = len(sched) - n_pf
        pf_tiles = []
        for j in range(n_pf):
            b, h, ti = sched[pf_start + j]
            w = P * (ti + 1)
            pf = cpool.tile([P, s], f32, name=f"pf{j}", tag=f"pf{j}")
            r0 = P * ti
            nc.sync.dma_start(out=pf[:, :w], in_=sc[b, h, r0 : r0 + P, 0:w])
            if w < s:
                nc.gpsimd.memset(pf[:, w:s], float("-inf"))
            pf_tiles.append(pf)

        n_slots = max(1, min(bufs, pf_start))
        slots = [
            cpool.tile([P, s], f32, name=f"t{j}", tag=f"t{j}") for j in range(n_slots)
        ]
        # first-use -inf tail fills via ACT copies from the const tile: big
        # gpsimd memsets steal SBUF AXI bandwidth from the DMA engines, and
        # DVE memsets delay the bias-strip chain / main adds — ACT is idle
        # until its write-issue duty starts.
        for i in range(n_slots):
            w0 = P * (sched[i][2] + 1)
            if w0 < s:
                nc.scalar.copy(slots[i][:, w0:s], inf_src[:, 0 : s - w0])
        for i, (b, h, ti) in enumerate(sched):
            w = P * (ti + 1)
            r0 = P * ti
            if i >= pf_start:
                t = pf_tiles[i - pf_start]
            else:
                t = slots[i % n_slots]
                nc.sync.dma_start(out=t[:, :w], in_=sc[b, h, r0 : r0 + P, 0:w])
            nc.vector.tensor_tensor(t[:, :w], t[:, :w], kts[h][:, s - w : s], add)
            nc.scalar.dma_start(out=out[b, h, r0 : r0 + P, 0:s], in_=t[:, 0:s])


def build_nc(bc, hc, s, bufs=8):
    import concourse.bacc as bacc
    import concourse.mybir as mybir
    import concourse.tile as tile

    f32 = mybir.dt.float32
    nc = bacc.Bacc("TRN2", target_bir_lowering=False, debug=False, num_devices=NCORES)
    io = {}
    io["scores"] = nc.declare_dram_parameter("scores", [bc, hc, s, s], f32, isOutput=False).ap()
    io["pvec"] = nc.declare_dram_parameter("pvec", [6 * hc], f32, isOutput=False).ap()
    io["pfull"] = nc.declare_dram_parameter("pfull", [P, s], f32, isOutput=False).ap()
    io["out"] = nc.declare_dram_parameter("out", [bc, hc, s, s], f32, isOutput=True).ap()

    with tile.TileContext(nc) as tc:
        build_kernel_body(tc, io, bc, hc, s, bufs=bufs)
    nc.compile()
    return nc


def make_in_maps(inputs, s=S):
    scores = np.ascontiguousarray(np.asarray(inputs["attention_scores"], dtype=np.float32))
    base = np.array(_base_slopes(H), dtype=np.float32)
    pfull = _const_tiles(s)
    in_maps = []
    for c in range(NCORES):
        hs = slice(c * HC, (c + 1) * HC)
        pvec = np.concatenate(
            [
                np.asarray(inputs["slope_scaling"], np.float32)[hs],
                np.asarray(inputs["sequential_bias"], np.float32)[hs],
                np.asarray(inputs["hierarchical_bias"], np.float32)[hs],
                base[hs],
                np.asarray(inputs["slope_adjustment"], np.float32)[hs],
                np.asarray(inputs["math_bias_scale"], np.float32)[hs],
            ]
        ).astype(np.float32)
        in_maps.append(
            {
                "scores": np.ascontiguousarray(scores[:, hs]),
                "pvec": pvec,
                "pfull": pfull,
            }
        )
    return in_maps


_NC_CACHE = {}


def _ensure_axon_hooks():
    """bass_utils imports antenv.axon_hooks when tracing is requested; this
    image's antenv lacks it.  Provide the documented shim so a profiling run
    works instead of crashing.  Never raises."""
    try:
        import importlib
        import sys
        import types

        try:
            importlib.import_module("antenv.axon_hooks")
            return
        except ImportError:
            pass
        import trn_agent_boot.trn_boot as tb

        hook = tb._ntff_profile_via_ctypes("/opt/axon/libaxon_pjrt.so")
        mod = types.ModuleType("antenv.axon_hooks")
        mod.get_axon_ntff_profile_hook = lambda: hook
        mod.set_axon_ntff_profile_hook = lambda h: None
        sys.modules["antenv.axon_hooks"] = mod
    except Exception:
        pass


def kernel(**inputs):
    _ensure_axon_hooks()
    from concourse.bass_utils import run_bass_kernel_spmd

    key = (B, HC, S)
    if key not in _NC_CACHE:
        _NC_CACHE[key] = build_nc(B, HC, S)
    nc = _NC_CACHE[key]
    in_maps = make_in_maps(inputs)
    res = run_bass_kernel_spmd(nc, in_maps, core_ids=list(range(NCORES)))
    out = np.empty((B, H, S, S), dtype=np.float32)
    for c in range(NCORES):
        out[:, c * HC : (c + 1) * HC] = res.results[c]["out"]
    return out


# revision 58
# speedup vs baseline: 1.3395x; 1.3395x over previous
"""ALiBi positional-bias kernel for 8 TRN2 NeuronCores.

out[b,h,i,j] = scores[b,h,i,j] + bias[h,i,j]
bias[h,i,j] = -inf                                   for j > i
            = d*A_h + B_h + P(d)*C_h  (d = i-j)      for j <= i
  A_h = (base_h*slope_scaling_h + slope_adjustment_h) * math_bias_scale_h
  B_h = sequential_bias_h
  C_h = hierarchical_bias_h
  P(d) = 0.1 (d in fib), 0.05 (d in pows), 0.02 (d%10==0), else 0

The bias is Toeplitz, so its 128x128 tiles depend only on the tile-diagonal
offset c = ti - tj: 16 distinct tiles per head.  Each core owns 2 heads x
2 batches; it builds the 16 bias tiles per head once in SBUF (as one
[128, 2048] strip, tile-diag c stored at column-tile 15-c so any causal row
strip is one contiguous slice), then streams score row-strips through a
single fused tensor_tensor add.  The strictly-upper-triangular region is
constant -inf and is written straight from a preset SBUF tile without ever
reading the scores there.
"""

import math

import numpy as np

B, H, S = 2, 16, 2048
P = 128
NT = S // P
NCORES = 8
HC = H // NCORES  # heads per core

VEC_NAMES = [
    "slope_adjustment",
    "slope_scaling",
    "math_bias_scale",
    "sequential_bias",
    "hierarchical_bias",
]


def _base_slopes(n):
    def pow2_slopes(m):
        start = 2 ** (-(2 ** (-(math.log2(m) - 3))))
        return [start * start**i for i in range(m)]

    if math.log2(n).is_integer():
        return pow2_slopes(n)
    c = 2 ** math.floor(math.log2(n))
    return pow2_slopes(c) + _base_slopes(2 * c)[0::2][: n - c]


def _hier_1d(n):
    d = np.arange(n)
    fib = np.array([1, 2, 3, 5, 8, 13, 21])
    pows = np.array([4, 16, 64, 256])
    return np.where(
        np.isin(d, fib), 0.1, np.where(np.isin(d, pows), 0.05, np.where(d % 10 == 0, 0.02, 0.0))
    ).astype(np.float32)


def _const_tiles(s):
    """pfull: [128, s] strip; column-tile m holds tile-diag c = nt-1-m,
    pfull[p, m*128+jj] = P(128*c + p - jj) (0 where d<0; -inf-masked later)."""
    nt = s // P
    hier = _hier_1d(s)
    p_idx = np.arange(P)[:, None]
    jj = np.arange(P)[None, :]
    pfull = np.zeros((P, s), np.float32)
    for m in range(nt):
        c = nt - 1 - m
        d = c * P + p_idx - jj
        pv = hier[np.clip(d, 0, s - 1)]
        pfull[:, m * P : (m + 1) * P] = np.where(d < 0, 0.0, pv)
    return pfull


def build_kernel_body(tc, io, bc, hc, s, bufs=8):
    """Emit the per-core program.  io maps tensor name -> bass.AP."""
    import concourse.mybir as mybir

    nc = tc.nc
    f32 = mybir.dt.float32
    add = mybir.AluOpType.add
    mult = mybir.AluOpType.mult
    nt = s // P

    with (
        tc.tile_pool(name="consts", bufs=1) as cpool,
        tc.tile_pool(name="work", bufs=2) as wpool,
    ):
        # -inf source for the slot-tail fills, first thing on the idle DVE
        inf_src = cpool.tile([P, s - P], f32, tag="inf_src")
        nc.vector.memset(inf_src[:], float("-inf"))

        # dfull[p, m*128+jj] = 128*(nt-1-m) + p - jj, generated on-device
        dfull = cpool.tile([P, s], f32, tag="dfull")
        nc.gpsimd.iota(
            dfull[:],
            pattern=[[-P, nt], [-1, P]],
            base=(nt - 1) * P,
            channel_multiplier=1,
            allow_small_or_imprecise_dtypes=True,
        )
        # warmup touch: absorbs the cold-start HBM receipt latency so the
        # pvec semaphore (which heads the bias-strip chain) fires sooner
        warm = cpool.tile([1, 4], f32, tag="warm")
        nc.sync.dma_start(out=warm[0:1, 0:4], in_=io["pfull"][0:1, 0:4])

        pfull = cpool.tile([P, s], f32, tag="pfull")
        nc.sync.dma_start(out=pfull[:], in_=io["pfull"])

        # packed params pvec = [scaling | seq | hier | base | adj | mscale], one
        # DMA; single_packet so the 48B transfer completes on one engine
        # instead of waiting for all 16 to cycle through
        ta = cpool.tile([1, 6 * hc], f32, tag="ta")
        nc.sync.dma_start(out=ta[0:1, :], in_=io["pvec"][None, :], single_packet=True)

        # abc0 = [A | B | C | ones(128)]; A = (scaling*base + adj) * mscale
        abc0 = cpool.tile([1, 3 * hc + P], f32, tag="abc0")
        nc.vector.tensor_tensor(
            abc0[0:1, 0:hc], ta[0:1, 0:hc], ta[0:1, 3 * hc : 4 * hc], mult
        )
        nc.vector.tensor_tensor(
            abc0[0:1, 0:hc], abc0[0:1, 0:hc], ta[0:1, 4 * hc : 5 * hc], add
        )
        nc.vector.tensor_tensor(
            abc0[0:1, 0:hc], abc0[0:1, 0:hc], ta[0:1, 5 * hc : 6 * hc], mult
        )
        nc.vector.tensor_copy(abc0[0:1, hc : 3 * hc], ta[0:1, hc : 3 * hc])
        nc.vector.memset(abc0[0:1, 3 * hc : 3 * hc + P], 1.0)

        # broadcast [1, 3*hc] -> [128, 3*hc] via ones outer-product on PE
        with tc.tile_pool(name="psum", bufs=1, space="PSUM") as ppool:
            pt = ppool.tile([P, 3 * hc], f32, tag="pt")
            nc.tensor.matmul(
                pt[:],
                abc0[0:1, 3 * hc : 3 * hc + P],
                abc0[0:1, 0 : 3 * hc],
                start=True,
                stop=True,
            )
            abcb = cpool.tile([P, 3 * hc], f32, tag="abcb")
            nc.vector.tensor_copy(abcb[:], pt[:])

        # bias strips per head: kt = D*A + (P*C + B), tile-diag c = nt-1-m
        # stored at column-tile m; row strip for row-tile ti = kt[:, s-w:s].
        kts = []
        for h in range(hc):
            kt = cpool.tile([P, s], f32, tag=f"k{h}")
            pc = wpool.tile([P, s], f32, tag="pc")
            nc.vector.tensor_scalar(
                pc[:],
                pfull[:],
                abcb[:, 2 * hc + h : 2 * hc + h + 1],
                abcb[:, hc + h : hc + h + 1],
                mult,
                add,
            )
            stt_inst = nc.vector.scalar_tensor_tensor(
                kt[:], dfull[:], abcb[:, h : h + 1], pc[:], mult, add
            )
            # diag tile: -inf where jj > p (strictly upper triangle)
            aff_inst = nc.gpsimd.affine_select(
                kt[:, s - P : s],
                kt[:, s - P : s],
                pattern=[[-1, P]],
                compare_op=mybir.AluOpType.is_ge,
                fill=float("-inf"),
                base=0,
                channel_multiplier=1,
            )
            kts.append(kt)

        sc = io["scores"]
        out = io["out"]
        # ti-major ascending: causal width w is globally non-decreasing, so
        # each slot's [w:s] tail keeps the -inf laid down at its first use —
        # every output write is one full-width DMA (uniform 8KB descriptors).
        # The tail's full-width reads are prefetched up front so the DMA
        # engines have work during the ramp while the bias strips are built.
        pairs = [(b, h) for h in range(hc) for b in range(bc)]  # h0 first
        sched = [(b, h, ti) for ti in range(nt) for (b, h) in pairs]

        n_pf = min(len(pairs), len(sched))  # prefetch the widest strips
        pf_start = len(sched) - n_pf
        pf_tiles = []
        for j in range(n_pf):
            b, h, ti = sched[pf_start + j]
            w = P * (ti + 1)
            pf = cpool.tile([P, s], f32, name=f"pf{j}", tag=f"pf{j}")
            r0 = P * ti
            nc.sync.dma_start(out=pf[:, :w], in_=sc[b, h, r0 : r0 + P, 0:w])
            if w < s:
                nc.gpsimd.memset(pf[:, w:s], float("-inf"))
            pf_tiles.append(pf)

        n_slots = max(1, min(bufs, pf_start))
        slots = [
            cpool.tile([P, s], f32, name=f"t{j}", tag=f"t{j}") for j in range(n_slots)
        ]
        # first-use -inf tail fills via ACT copies from the const tile: big
        # gpsimd memsets steal SBUF AXI bandwidth from the DMA engines, and
        # DVE memsets delay the bias-strip chain / main adds — ACT is idle
        # until its write-issue duty starts.
        for i in range(n_slots):
            w0 = P * (sched[i][2] + 1)
            if w0 < s:
                nc.scalar.copy(slots[i][:, w0:s], inf_src[:, 0 : s - w0])
        for i, (b, h, ti) in enumerate(sched):
            w = P * (ti + 1)
            r0 = P * ti
            if i >= pf_start:
                t = pf_tiles[i - pf_start]
            else:
                t = slots[i % n_slots]
                nc.sync.dma_start(out=t[:, :w], in_=sc[b, h, r0 : r0 + P, 0:w])
            nc.vector.tensor_tensor(t[:, :w], t[:, :w], kts[h][:, s - w : s], add)
            nc.scalar.dma_start(out=out[b, h, r0 : r0 + P, 0:s], in_=t[:, 0:s])


def build_nc(bc, hc, s, bufs=8):
    import concourse.bacc as bacc
    import concourse.mybir as mybir
    import concourse.tile as tile

    f32 = mybir.dt.float32
    nc = bacc.Bacc("TRN2", target_bir_lowering=False, debug=False, num_devices=NCORES)
    io = {}
    io["scores"] = nc.declare_dram_parameter("scores", [bc, hc, s, s], f32, isOutput=False).ap()
    io["pvec"] = nc.declare_dram_parameter("pvec", [6 * hc], f32, isOutput=False).ap()
    io["pfull"] = nc.declare_dram_parameter("pfull", [P, s], f32, isOutput=False).ap()
    io["out"] = nc.declare_dram_parameter("out", [bc, hc, s, s], f32, isOutput=True).ap()

    with tile.TileContext(nc) as tc:
        build_kernel_body(tc, io, bc, hc, s, bufs=bufs)
    nc.compile()
    return nc


def make_in_maps(inputs, s=S):
    scores = np.ascontiguousarray(np.asarray(inputs["attention_scores"], dtype=np.float32))
    base = np.array(_base_slopes(H), dtype=np.float32)
    pfull = _const_tiles(s)
    in_maps = []
    for c in range(NCORES):
        hs = slice(c * HC, (c + 1) * HC)
        pvec = np.concatenate(
            [
                np.asarray(inputs["slope_scaling"], np.float32)[hs],
                np.asarray(inputs["sequential_bias"], np.float32)[hs],
                np.asarray(inputs["hierarchical_bias"], np.float32)[hs],
                base[hs],
                np.asarray(inputs["slope_adjustment"], np.float32)[hs],
                np.asarray(inputs["math_bias_scale"], np.float32)[hs],
            ]
        ).astype(np.float32)
        in_maps.append(
            {
                "scores": np.ascontiguousarray(scores[:, hs]),
                "pvec": pvec,
                "pfull": pfull,
            }
        )
    return in_maps


_NC_CACHE = {}


def _ensure_axon_hooks():
    """bass_utils imports antenv.axon_hooks when tracing is requested; this
    image's antenv lacks it.  Provide the documented shim so a profiling run
    works instead of crashing.  Never raises."""
    try:
        import importlib
        import sys
        import types

        try:
            importlib.import_module("antenv.axon_hooks")
            return
        except ImportError:
            pass
        import trn_agent_boot.trn_boot as tb

        hook = tb._ntff_profile_via_ctypes("/opt/axon/libaxon_pjrt.so")
        mod = types.ModuleType("antenv.axon_hooks")
        mod.get_axon_ntff_profile_hook = lambda: hook
        mod.set_axon_ntff_profile_hook = lambda h: None
        sys.modules["antenv.axon_hooks"] = mod
    except Exception:
        pass


def kernel(**inputs):
    _ensure_axon_hooks()
    from concourse.bass_utils import run_bass_kernel_spmd

    key = (B, HC, S)
    if key not in _NC_CACHE:
        _NC_CACHE[key] = build_nc(B, HC, S)
    nc = _NC_CACHE[key]
    in_maps = make_in_maps(inputs)
    res = run_bass_kernel_spmd(nc, in_maps, core_ids=list(range(NCORES)))
    out = np.empty((B, H, S, S), dtype=np.float32)
    for c in range(NCORES):
        out[:, c * HC : (c + 1) * HC] = res.results[c]["out"]
    return out


# revision 61
# speedup vs baseline: 1.3523x; 1.0095x over previous
"""ALiBi positional-bias kernel for 8 TRN2 NeuronCores.

out[b,h,i,j] = scores[b,h,i,j] + bias[h,i,j]
bias[h,i,j] = -inf                                   for j > i
            = d*A_h + B_h + P(d)*C_h  (d = i-j)      for j <= i
  A_h = (base_h*slope_scaling_h + slope_adjustment_h) * math_bias_scale_h
  B_h = sequential_bias_h
  C_h = hierarchical_bias_h
  P(d) = 0.1 (d in fib), 0.05 (d in pows), 0.02 (d%10==0), else 0

The bias is Toeplitz, so its 128x128 tiles depend only on the tile-diagonal
offset c = ti - tj: 16 distinct tiles per head.  Each core owns 2 heads x
2 batches; it builds the 16 bias tiles per head once in SBUF (as one
[128, 2048] strip, tile-diag c stored at column-tile 15-c so any causal row
strip is one contiguous slice), then streams score row-strips through a
single fused tensor_tensor add.  The strictly-upper-triangular region is
constant -inf and is written straight from a preset SBUF tile without ever
reading the scores there.
"""

import math

import numpy as np

B, H, S = 2, 16, 2048
P = 128
NT = S // P
NCORES = 8
HC = H // NCORES  # heads per core

VEC_NAMES = [
    "slope_adjustment",
    "slope_scaling",
    "math_bias_scale",
    "sequential_bias",
    "hierarchical_bias",
]


def _base_slopes(n):
    def pow2_slopes(m):
        start = 2 ** (-(2 ** (-(math.log2(m) - 3))))
        return [start * start**i for i in range(m)]

    if math.log2(n).is_integer():
        return pow2_slopes(n)
    c = 2 ** math.floor(math.log2(n))
    return pow2_slopes(c) + _base_slopes(2 * c)[0::2][: n - c]


def _hier_1d(n):
    d = np.arange(n)
    fib = np.array([1, 2, 3, 5, 8, 13, 21])
    pows = np.array([4, 16, 64, 256])
    return np.where(
        np.isin(d, fib), 0.1, np.where(np.isin(d, pows), 0.05, np.where(d % 10 == 0, 0.02, 0.0))
    ).astype(np.float32)


def _const_tiles(s):
    """pfull: [128, s] strip; column-tile m holds tile-diag c = nt-1-m,
    pfull[p, m*128+jj] = P(128*c + p - jj) (0 where d<0; -inf-masked later)."""
    nt = s // P
    hier = _hier_1d(s)
    p_idx = np.arange(P)[:, None]
    jj = np.arange(P)[None, :]
    pfull = np.zeros((P, s), np.float32)
    for m in range(nt):
        c = nt - 1 - m
        d = c * P + p_idx - jj
        pv = hier[np.clip(d, 0, s - 1)]
        pfull[:, m * P : (m + 1) * P] = np.where(d < 0, 0.0, pv)
    return pfull


def build_kernel_body(tc, io, bc, hc, s, bufs=8):
    """Emit the per-core program.  io maps tensor name -> bass.AP."""
    import concourse.mybir as mybir

    nc = tc.nc
    f32 = mybir.dt.float32
    add = mybir.AluOpType.add
    mult = mybir.AluOpType.mult
    nt = s // P

    with (
        tc.tile_pool(name="consts", bufs=1) as cpool,
        tc.tile_pool(name="work", bufs=2) as wpool,
    ):
        # -inf source for the slot-tail fills, first thing on the idle DVE
        inf_src = cpool.tile([P, s - P], f32, tag="inf_src")
        nc.vector.memset(inf_src[:], float("-inf"))

        # dfull[p, m*128+jj] = 128*(nt-1-m) + p - jj, generated on-device
        dfull = cpool.tile([P, s], f32, tag="dfull")
        nc.gpsimd.iota(
            dfull[:],
            pattern=[[-P, nt], [-1, P]],
            base=(nt - 1) * P,
            channel_multiplier=1,
            allow_small_or_imprecise_dtypes=True,
        )
        # packed params pvec = [scaling | seq | hier | base | adj | mscale]:
        # FIRST DMA, single_packet, and the prefetch reads are gated behind
        # its semaphore so its HBM receipt isn't delayed by competing traffic
        # (it heads the bias-strip critical path)
        ta = cpool.tile([1, 6 * hc], f32, tag="ta")
        pvec_dma = nc.sync.dma_start(
            out=ta[0:1, :], in_=io["pvec"][None, :], single_packet=True
        )

        pfull = cpool.tile([P, s], f32, tag="pfull")
        nc.sync.dma_start(out=pfull[:], in_=io["pfull"])

        # abc0 = [A | B | C | ones(128)]; A = (scaling*base + adj) * mscale
        abc0 = cpool.tile([1, 3 * hc + P], f32, tag="abc0")
        nc.vector.tensor_tensor(
            abc0[0:1, 0:hc], ta[0:1, 0:hc], ta[0:1, 3 * hc : 4 * hc], mult
        )
        nc.vector.tensor_tensor(
            abc0[0:1, 0:hc], abc0[0:1, 0:hc], ta[0:1, 4 * hc : 5 * hc], add
        )
        nc.vector.tensor_tensor(
            abc0[0:1, 0:hc], abc0[0:1, 0:hc], ta[0:1, 5 * hc : 6 * hc], mult
        )
        nc.vector.tensor_copy(abc0[0:1, hc : 3 * hc], ta[0:1, hc : 3 * hc])
        nc.vector.memset(abc0[0:1, 3 * hc : 3 * hc + P], 1.0)

        # broadcast [1, 3*hc] -> [128, 3*hc] via ones outer-product on PE
        with tc.tile_pool(name="psum", bufs=1, space="PSUM") as ppool:
            pt = ppool.tile([P, 3 * hc], f32, tag="pt")
            nc.tensor.matmul(
                pt[:],
                abc0[0:1, 3 * hc : 3 * hc + P],
                abc0[0:1, 0 : 3 * hc],
                start=True,
                stop=True,
            )
            abcb = cpool.tile([P, 3 * hc], f32, tag="abcb")
            nc.vector.tensor_copy(abcb[:], pt[:])

        # bias strips per head: kt = D*A + (P*C + B), tile-diag c = nt-1-m
        # stored at column-tile m; row strip for row-tile ti = kt[:, s-w:s].
        kts = []
        for h in range(hc):
            kt = cpool.tile([P, s], f32, tag=f"k{h}")
            pc = wpool.tile([P, s], f32, tag="pc")
            nc.vector.tensor_scalar(
                pc[:],
                pfull[:],
                abcb[:, 2 * hc + h : 2 * hc + h + 1],
                abcb[:, hc + h : hc + h + 1],
                mult,
                add,
            )
            stt_inst = nc.vector.scalar_tensor_tensor(
                kt[:], dfull[:], abcb[:, h : h + 1], pc[:], mult, add
            )
            # diag tile: -inf where jj > p (strictly upper triangle)
            aff_inst = nc.gpsimd.affine_select(
                kt[:, s - P : s],
                kt[:, s - P : s],
                pattern=[[-1, P]],
                compare_op=mybir.AluOpType.is_ge,
                fill=float("-inf"),
                base=0,
                channel_multiplier=1,
            )
            kts.append(kt)

        sc = io["scores"]
        out = io["out"]
        # ti-major ascending: causal width w is globally non-decreasing, so
        # each slot's [w:s] tail keeps the -inf laid down at its first use —
        # every output write is one full-width DMA (uniform 8KB descriptors).
        # The tail's full-width reads are prefetched up front so the DMA
        # engines have work during the ramp while the bias strips are built.
        pairs = [(b, h) for h in range(hc) for b in range(bc)]  # h0 first
        sched = [(b, h, ti) for ti in range(nt) for (b, h) in pairs]

        from concourse.tile import add_dep_helper

        n_pf = min(len(pairs) + 2, len(sched))  # prefetch the widest strips
        pf_start = len(sched) - n_pf
        pf_tiles = []
        for j in range(n_pf):
            b, h, ti = sched[pf_start + j]
            w = P * (ti + 1)
            pf = cpool.tile([P, s], f32, name=f"pf{j}", tag=f"pf{j}")
            r0 = P * ti
            d = nc.sync.dma_start(out=pf[:, :w], in_=sc[b, h, r0 : r0 + P, 0:w])
            add_dep_helper(d.ins, pvec_dma.ins, sync=True, reason="pf after pvec")
            if w < s:
                nc.scalar.copy(pf[:, w:s], inf_src[:, 0 : s - w])
            pf_tiles.append(pf)

        n_slots = max(1, min(bufs, pf_start))
        slots = [
            cpool.tile([P, s], f32, name=f"t{j}", tag=f"t{j}") for j in range(n_slots)
        ]
        # first-use -inf tail fills via ACT copies from the const tile: big
        # gpsimd memsets steal SBUF AXI bandwidth from the DMA engines, and
        # DVE memsets delay the bias-strip chain / main adds — ACT is idle
        # until its write-issue duty starts.
        for i in range(n_slots):
            w0 = P * (sched[i][2] + 1)
            if w0 < s:
                nc.scalar.copy(slots[i][:, w0:s], inf_src[:, 0 : s - w0])
        for i, (b, h, ti) in enumerate(sched):
            w = P * (ti + 1)
            r0 = P * ti
            if i >= pf_start:
                t = pf_tiles[i - pf_start]
            else:
                t = slots[i % n_slots]
                nc.sync.dma_start(out=t[:, :w], in_=sc[b, h, r0 : r0 + P, 0:w])
            nc.vector.tensor_tensor(t[:, :w], t[:, :w], kts[h][:, s - w : s], add)
            nc.scalar.dma_start(out=out[b, h, r0 : r0 + P, 0:s], in_=t[:, 0:s])


def build_nc(bc, hc, s, bufs=8):
    import concourse.bacc as bacc
    import concourse.mybir as mybir
    import concourse.tile as tile

    f32 = mybir.dt.float32
    nc = bacc.Bacc("TRN2", target_bir_lowering=False, debug=False, num_devices=NCORES)
    io = {}
    io["scores"] = nc.declare_dram_parameter("scores", [bc, hc, s, s], f32, isOutput=False).ap()
    io["pvec"] = nc.declare_dram_parameter("pvec", [6 * hc], f32, isOutput=False).ap()
    io["pfull"] = nc.declare_dram_parameter("pfull", [P, s], f32, isOutput=False).ap()
    io["out"] = nc.declare_dram_parameter("out", [bc, hc, s, s], f32, isOutput=True).ap()

    with tile.TileContext(nc) as tc:
        build_kernel_body(tc, io, bc, hc, s, bufs=bufs)
    nc.compile()
    return nc


def make_in_maps(inputs, s=S):
    scores = np.ascontiguousarray(np.asarray(inputs["attention_scores"], dtype=np.float32))
    base = np.array(_base_slopes(H), dtype=np.float32)
    pfull = _const_tiles(s)
    in_maps = []
    for c in range(NCORES):
        hs = slice(c * HC, (c + 1) * HC)
        pvec = np.concatenate(
            [
                np.asarray(inputs["slope_scaling"], np.float32)[hs],
                np.asarray(inputs["sequential_bias"], np.float32)[hs],
                np.asarray(inputs["hierarchical_bias"], np.float32)[hs],
                base[hs],
                np.asarray(inputs["slope_adjustment"], np.float32)[hs],
                np.asarray(inputs["math_bias_scale"], np.float32)[hs],
            ]
        ).astype(np.float32)
        in_maps.append(
            {
                "scores": np.ascontiguousarray(scores[:, hs]),
                "pvec": pvec,
                "pfull": pfull,
            }
        )
    return in_maps


_NC_CACHE = {}


def _ensure_axon_hooks():
    """bass_utils imports antenv.axon_hooks when tracing is requested; this
    image's antenv lacks it.  Provide the documented shim so a profiling run
    works instead of crashing.  Never raises."""
    try:
        import importlib
        import sys
        import types

        try:
            importlib.import_module("antenv.axon_hooks")
            return
        except ImportError:
            pass
        import trn_agent_boot.trn_boot as tb

        hook = tb._ntff_profile_via_ctypes("/opt/axon/libaxon_pjrt.so")
        mod = types.ModuleType("antenv.axon_hooks")
        mod.get_axon_ntff_profile_hook = lambda: hook
        mod.set_axon_ntff_profile_hook = lambda h: None
        sys.modules["antenv.axon_hooks"] = mod
    except Exception:
        pass


def kernel(**inputs):
    _ensure_axon_hooks()
    from concourse.bass_utils import run_bass_kernel_spmd

    key = (B, HC, S)
    if key not in _NC_CACHE:
        _NC_CACHE[key] = build_nc(B, HC, S)
    nc = _NC_CACHE[key]
    in_maps = make_in_maps(inputs)
    res = run_bass_kernel_spmd(nc, in_maps, core_ids=list(range(NCORES)))
    out = np.empty((B, H, S, S), dtype=np.float32)
    for c in range(NCORES):
        out[:, c * HC : (c + 1) * HC] = res.results[c]["out"]
    return out
